# revision 9
# baseline (speedup 1.0000x reference)
"""Multi-head attention (B=4,S=2048,H=1024,NH=16,D=64) on 8 trn2 cores.

Sharding: core c = (g, b) with g = c // 4 (head-group of 8 heads = 512 dims,
tensor parallel) and b = c % 4 (batch, data parallel). Each core computes a
partial output (its head-group's contribution to the final projection),
transposed: ot = (attn_out_g @ wo_g)^T of shape [H, S]. Host sums the two
group partials per batch and adds bias.

Math notes (host/device split):
  - k-proj bias bk drops out of softmax (adds a per-query constant along the
    key axis), so it is not applied on device.
  - v-proj bias bv commutes through normalized attention (rows of the score
    matrix sum to 1): its contribution is bv @ wo, folded into the output
    bias on the host.

On-device layout: everything is computed transposed (feature dim on
partitions, sequence on the free axis) so the softmax key-axis lands on
partitions. Scores S^T are built per head as K_h^T(stationary) x Q_h^T,
exp() runs on the scalar engine straight out of PSUM, and the ones-column
appended to V in the AV matmul yields the softmax denominators for free.
"""

import sys

if "/opt/trn_rl_repo" not in sys.path:
    sys.path.insert(0, "/opt/trn_rl_repo")

import numpy as np

B, S, H, NH, D = 4, 2048, 1024, 16, 64
G = 2  # head-group split across cores (tensor parallel axis)
GH = H // G  # 512 dims (8 heads) per group
NCORES = 8
SCALE = 1.0 / float(D) ** 0.5  # 1/8

KT = H // 128  # 8 contraction tiles for projections
MT = GH // 128  # 4 m-tiles = head pairs per group
NQC = S // 512  # 4 sequence chunks of 512
SQ = S // 128  # 16 key-sequence tiles
VW = D + 1  # 65: V columns + ones column per head

_CACHE = {}

# build-time tuning knobs (TimelineSim-swept)
CFG = {
    "xs_bufs": 16,
    "w_bufs": 10,
    "pt_bufs": 8,
    "mm_bufs": 2,
    "o_bufs": 2,
}


def _build():
    import concourse.tile as tile
    from concourse import bacc, mybir

    F32 = mybir.dt.float32
    F32R = mybir.dt.float32r
    BF16 = mybir.dt.float16  # f16: same PE speed as bf16, 3 more mantissa bits
    AF = mybir.ActivationFunctionType
    OP = mybir.AluOpType

    nc = bacc.Bacc("TRN2", target_bir_lowering=False, debug=False)

    xq = nc.dram_tensor("xq", [H, S], F32R, kind="ExternalInput")
    xk = nc.dram_tensor("xk", [H, S], F32R, kind="ExternalInput")
    xv = nc.dram_tensor("xv", [H, S], F32R, kind="ExternalInput")
    wqd = nc.dram_tensor("wq", [H, GH], F32R, kind="ExternalInput")
    wkd = nc.dram_tensor("wk", [H, GH], F32R, kind="ExternalInput")
    wvd = nc.dram_tensor("wv", [H, GH], F32R, kind="ExternalInput")
    wod = nc.dram_tensor("wo", [GH, H], F32, kind="ExternalInput")
    bqd = nc.dram_tensor("bq", [GH], F32, kind="ExternalInput")
    otd = nc.dram_tensor("ot", [H, S], F32, kind="ExternalOutput")

    with tile.TileContext(nc) as tc:
        with (
            tc.tile_pool(name="res", bufs=1) as res,
            tc.tile_pool(name="rot", bufs=2) as rot,
            tc.tile_pool(name="psmm", bufs=CFG["mm_bufs"], space="PSUM") as psmm,
            tc.tile_pool(name="pso", bufs=CFG["o_bufs"], space="PSUM") as pso,
            tc.tile_pool(name="dsc", bufs=4, space="DRAM") as dsc,
        ):
            # ---- residents ----
            qhT = [
                res.tile([128, S], F32R, tag=f"qhT{m}", name=f"qhT{m}")
                for m in range(MT)
            ]
            khT = [
                res.tile([128, S], F32R, tag=f"khT{m}", name=f"khT{m}")
                for m in range(MT)
            ]
            oT = [
                res.tile([128, S], BF16, tag=f"oT{t}", name=f"oT{t}")
                for t in range(MT)
            ]
            vaug = res.tile([128, SQ * 8 * VW], BF16, tag="vaug", name="vaug")
            wo_bf = [
                res.tile([128, H], BF16, tag=f"wob{t}", name=f"wob{t}")
                for t in range(MT)
            ]
            bq_sb = res.tile([128, MT], F32, tag="bqsb", name="bq_sb")

            # ---- constants / weights staging ----
            for m in range(MT):
                nc.sync.dma_start(
                    out=bq_sb[:, m : m + 1],
                    in_=bqd.ap()[m * 128 : (m + 1) * 128].rearrange(
                        "(p o) -> p o", o=1
                    ),
                )
            for t in range(MT):
                wos = rot.tile([128, H], F32, tag="wos", bufs=2, name=f"wos{t}")
                nc.sync.dma_start(out=wos, in_=wod.ap()[t * 128 : (t + 1) * 128, :])
                nc.vector.tensor_copy(wo_bf[t], wos)
            # ones columns of vaug (V slots are overwritten by the V proj)
            nc.vector.memset(vaug, 1.0)

            def load_w(wd):
                ws = []
                for kt in range(KT):
                    wt = rot.tile([128, GH], F32R, tag="w", bufs=CFG["w_bufs"], name=f"w{kt}")
                    nc.sync.dma_start(out=wt, in_=wd.ap()[kt * 128 : (kt + 1) * 128, :])
                    ws.append(wt)
                return ws

            def load_strips(xd, qc):
                xs = []
                for kt in range(KT):
                    st = rot.tile([128, 512], F32R, tag="xs", bufs=CFG["xs_bufs"], name=f"xs{kt}")
                    nc.sync.dma_start(
                        out=st,
                        in_=xd.ap()[
                            kt * 128 : (kt + 1) * 128, qc * 512 : (qc + 1) * 512
                        ],
                    )
                    xs.append(st)
                return xs

            # ---- V projection: vaug[kseq, head*65] (untransposed, bf16) ----
            wv_sb = load_w(wvd)
            for qc in range(NQC):
                xs = load_strips(xv, qc)
                for sql in range(4):
                    sq = qc * 4 + sql
                    ps = psmm.tile([128, 512], F32, tag="mm", name=f"psv{sq}")
                    for kt in range(KT):
                        nc.tensor.matmul(
                            ps,
                            lhsT=xs[kt][:, sql * 128 : (sql + 1) * 128],
                            rhs=wv_sb[kt],
                            start=(kt == 0),
                            stop=(kt == KT - 1),
                        )
                    base = sq * 8 * VW
                    for h in range(8):
                        nc.vector.tensor_copy(
                            vaug[:, base + h * VW : base + h * VW + D],
                            ps[:, h * D : (h + 1) * D],
                        )

            # ---- K projection: khT[m] = (xk @ wk)^T slice, f32r ----
            wk_sb = load_w(wkd)
            for qc in range(NQC):
                xs = load_strips(xk, qc)
                for m in range(MT):
                    ps = psmm.tile([128, 512], F32, tag="mm", name=f"psk{m}")
                    for kt in range(KT):
                        nc.tensor.matmul(
                            ps,
                            lhsT=wk_sb[kt][:, m * 128 : (m + 1) * 128],
                            rhs=xs[kt],
                            start=(kt == 0),
                            stop=(kt == KT - 1),
                        )
                    nc.vector.tensor_copy(khT[m][:, qc * 512 : (qc + 1) * 512], ps)

            # ---- Q projection (+ bias) ----
            wq_sb = load_w(wqd)

            def q_proj(qc):
                xs = load_strips(xq, qc)
                for m in range(MT):
                    ps = psmm.tile([128, 512], F32, tag="mm", name=f"psq{m}")
                    for kt in range(KT):
                        nc.tensor.matmul(
                            ps,
                            lhsT=wq_sb[kt][:, m * 128 : (m + 1) * 128],
                            rhs=xs[kt],
                            start=(kt == 0),
                            stop=(kt == KT - 1),
                        )
                    nc.vector.tensor_scalar(
                        qhT[m][:, qc * 512 : (qc + 1) * 512],
                        ps,
                        bq_sb[:, m : m + 1],
                        None,
                        OP.add,
                    )

            # ---- attention for head pair t over a 1024-wide query chunk ----
            def attention(t, qcp):
                q0 = qcp * 1024
                ps_o = [
                    pso.tile([VW, 1024], F32, tag="o", name=f"pso{hh}")
                    for hh in range(2)
                ]
                for kt in range(SQ):
                    # emit the two heads' score matmuls adjacently: they hit
                    # disjoint PE row groups (partitions 0-63 vs 64-127) and
                    # run concurrently on hardware (row tiling)
                    ps_ss = []
                    for hh in range(2):
                        hp = 64 * hh
                        ps_s = psmm.tile([128, 1024], F32, tag="mm", name="pss")
                        ps_ss.append(ps_s)
                        for qch in range(2):
                            nc.tensor.matmul(
                                ps_s[:, qch * 512 : (qch + 1) * 512],
                                lhsT=khT[t][hp : hp + 64, kt * 128 : (kt + 1) * 128],
                                rhs=qhT[t][
                                    hp : hp + 64, q0 + qch * 512 : q0 + (qch + 1) * 512
                                ],
                                start=True,
                                stop=True,
                            )
                    for hh in range(2):
                        h_abs = 2 * t + hh
                        pt_t = rot.tile([128, 1024], BF16, tag="pt", bufs=CFG["pt_bufs"], name="pt")
                        nc.scalar.activation(pt_t, ps_ss[hh], AF.Exp, scale=SCALE)
                        vbase = kt * 8 * VW + h_abs * VW
                        for qch in range(2):
                            nc.tensor.matmul(
                                ps_o[hh][:, qch * 512 : (qch + 1) * 512],
                                lhsT=vaug[:, vbase : vbase + VW],
                                rhs=pt_t[:, qch * 512 : (qch + 1) * 512],
                                start=(kt == 0),
                                stop=(kt == SQ - 1),
                            )
                # normalize by the ones-column sums; heads stack on partitions
                for hh in range(2):
                    rcp = rot.tile([VW, 1024], F32, tag="rcp", bufs=2, name="rcp")
                    nc.vector.reciprocal(rcp[D : D + 1, :], ps_o[hh][D : D + 1, :])
                    # broadcast the recip row to 64 partitions via a DRAM
                    # round-trip (DRAM source DMA supports 0-stride partition
                    # reads; SBUF source / gpsimd custom op do not work here)
                    sc = dsc.tile([1, 1024], F32, tag="sc", name="sc")
                    nc.sync.dma_start(out=sc, in_=rcp[D : D + 1, :])
                    bc = rot.tile([VW, 1024], F32, tag="bc", bufs=2, name="bc")
                    nc.sync.dma_start(
                        out=bc[0:64, :], in_=sc[0, :].partition_broadcast(64)
                    )
                    if hh == 0:
                        nc.vector.tensor_tensor(
                            oT[t][0:64, q0 : q0 + 1024],
                            ps_o[0][0:D, :],
                            bc[0:64, :],
                            OP.mult,
                        )
                    else:
                        # normalized h1 lands on partitions 0-63; DMA shifts it
                        # onto partitions 64-127 of the head-pair tile
                        otn = rot.tile([64, 1024], BF16, tag="otn", bufs=2, name="otn")
                        nc.vector.tensor_tensor(
                            otn, ps_o[1][0:D, :], bc[0:64, :], OP.mult
                        )
                        nc.sync.dma_start(
                            out=oT[t][64:128, q0 : q0 + 1024], in_=otn
                        )

            # ---- output projection for one 512-wide sequence chunk ----
            def out_proj(qcc):
                for m in range(H // 128):
                    ps = psmm.tile([128, 512], F32, tag="mm", name=f"pso{m}")
                    for t in range(MT):
                        nc.tensor.matmul(
                            ps,
                            lhsT=wo_bf[t][:, m * 128 : (m + 1) * 128],
                            rhs=oT[t][:, qcc * 512 : (qcc + 1) * 512],
                            start=(t == 0),
                            stop=(t == MT - 1),
                        )
                    osb = rot.tile([128, 512], F32, tag="osb", bufs=3, name="osb")
                    nc.vector.tensor_copy(osb, ps)
                    nc.sync.dma_start(
                        out=otd.ap()[m * 128 : (m + 1) * 128, qcc * 512 : (qcc + 1) * 512],
                        in_=osb,
                    )

            for qcp in range(2):
                q_proj(2 * qcp)
                q_proj(2 * qcp + 1)
                for t in range(MT):
                    attention(t, qcp)
                out_proj(2 * qcp)
                out_proj(2 * qcp + 1)

    nc.compile()
    return nc


def _get_nc():
    if "nc" not in _CACHE:
        _CACHE["nc"] = _build()
    return _CACHE["nc"]


def make_in_maps(q, k, v, wq, wk, wv, wo, bq):
    q = np.asarray(q, np.float32)
    k = np.asarray(k, np.float32)
    v = np.asarray(v, np.float32)
    in_maps = []
    for c in range(NCORES):
        g, b = divmod(c, B)
        sl = slice(g * GH, (g + 1) * GH)
        in_maps.append(
            {
                "xq": np.ascontiguousarray(q[b].T),
                "xk": np.ascontiguousarray(k[b].T),
                "xv": np.ascontiguousarray(v[b].T),
                "wq": np.ascontiguousarray(np.asarray(wq, np.float32)[:, sl]),
                "wk": np.ascontiguousarray(np.asarray(wk, np.float32)[:, sl]),
                "wv": np.ascontiguousarray(np.asarray(wv, np.float32)[:, sl]),
                "wo": np.ascontiguousarray(np.asarray(wo, np.float32)[sl, :]),
                "bq": np.ascontiguousarray(np.asarray(bq, np.float32)[sl]),
            }
        )
    return in_maps


def assemble(per_core_ot, bv, wo, bo):
    bo_eff = (
        np.asarray(bo, np.float32)
        + np.asarray(bv, np.float32) @ np.asarray(wo, np.float32)
    )
    out = np.empty((B, S, H), np.float32)
    for b in range(B):
        out[b] = per_core_ot[b].T + per_core_ot[B + b].T + bo_eff
    return out


def kernel(q, k, v, wq, bq, wk, bk, wv, bv, wo, bo, _trace=False):
    from concourse.bass_utils import run_bass_kernel_spmd

    nc = _get_nc()
    in_maps = make_in_maps(q, k, v, wq, wk, wv, wo, bq)
    res = run_bass_kernel_spmd(
        nc, in_maps, core_ids=list(range(NCORES)), trace=_trace
    )
    _CACHE["last_results"] = res
    outs = [res.results[c]["ot"] for c in range(NCORES)]
    return assemble(outs, bv, wo, bo)


# revision 10
# speedup vs baseline: 2.8951x; 2.8951x over previous
"""Multi-head attention (B=4,S=2048,H=1024,NH=16,D=64) on 8 trn2 cores.

Sharding: core c = (g, b) with g = c // 4 (head-group of 8 heads = 512 dims,
tensor parallel) and b = c % 4 (batch, data parallel). Each core computes a
partial output (its head-group's contribution to the final projection),
transposed: ot = (attn_out_g @ wo_g)^T of shape [H, S]. Host sums the two
group partials per batch and adds bias.

Math notes (host/device split):
  - k-proj bias bk drops out of softmax (adds a per-query constant along the
    key axis), so it is not applied on device.
  - v-proj bias bv commutes through normalized attention (rows of the score
    matrix sum to 1): its contribution is bv @ wo, folded into the output
    bias on the host.

On-device layout: everything is computed transposed (feature dim on
partitions, sequence on the free axis) so the softmax key-axis lands on
partitions. Scores S^T are built per head as K_h^T(stationary) x Q_h^T,
exp() runs on the scalar engine straight out of PSUM, and the ones-column
appended to V in the AV matmul yields the softmax denominators for free.
"""

import sys

if "/opt/trn_rl_repo" not in sys.path:
    sys.path.insert(0, "/opt/trn_rl_repo")

import numpy as np

B, S, H, NH, D = 4, 2048, 1024, 16, 64
G = 2  # head-group split across cores (tensor parallel axis)
GH = H // G  # 512 dims (8 heads) per group
NCORES = 8
SCALE = 1.0 / float(D) ** 0.5  # 1/8

KT = H // 128  # 8 contraction tiles for projections
MT = GH // 128  # 4 m-tiles = head pairs per group
NQC = S // 512  # 4 sequence chunks of 512
SQ = S // 128  # 16 key-sequence tiles
VW = D + 1  # 65: V columns + ones column per head

_CACHE = {}

# build-time tuning knobs (TimelineSim-swept)
CFG = {
    "xs_bufs": 16,
    "w_bufs": 10,
    "pt_bufs": 8,
    "mm_bufs": 2,
    "o_bufs": 2,
}


def _build():
    import concourse.tile as tile
    from concourse import bacc, mybir

    F32 = mybir.dt.float32
    F32R = mybir.dt.float16  # all-f16 variant: f16 matmuls everywhere
    BF16 = mybir.dt.float16  # f16: same PE speed as bf16, 3 more mantissa bits
    AF = mybir.ActivationFunctionType
    OP = mybir.AluOpType

    nc = bacc.Bacc("TRN2", target_bir_lowering=False, debug=False)

    xq = nc.dram_tensor("xq", [H, S], F32R, kind="ExternalInput")
    xk = nc.dram_tensor("xk", [H, S], F32R, kind="ExternalInput")
    xv = nc.dram_tensor("xv", [H, S], F32R, kind="ExternalInput")
    wqd = nc.dram_tensor("wq", [H, GH], F32R, kind="ExternalInput")
    wkd = nc.dram_tensor("wk", [H, GH], F32R, kind="ExternalInput")
    wvd = nc.dram_tensor("wv", [H, GH], F32R, kind="ExternalInput")
    wod = nc.dram_tensor("wo", [GH, H], F32, kind="ExternalInput")
    bqd = nc.dram_tensor("bq", [GH], F32, kind="ExternalInput")
    otd = nc.dram_tensor("ot", [H, S], F32, kind="ExternalOutput")

    with tile.TileContext(nc) as tc:
        with (
            tc.tile_pool(name="res", bufs=1) as res,
            tc.tile_pool(name="rot", bufs=2) as rot,
            tc.tile_pool(name="psmm", bufs=CFG["mm_bufs"], space="PSUM") as psmm,
            tc.tile_pool(name="pso", bufs=CFG["o_bufs"], space="PSUM") as pso,
            tc.tile_pool(name="dsc", bufs=4, space="DRAM") as dsc,
        ):
            # ---- residents ----
            qhT = [
                res.tile([128, S], F32R, tag=f"qhT{m}", name=f"qhT{m}")
                for m in range(MT)
            ]
            khT = [
                res.tile([128, S], F32R, tag=f"khT{m}", name=f"khT{m}")
                for m in range(MT)
            ]
            oT = [
                res.tile([128, S], BF16, tag=f"oT{t}", name=f"oT{t}")
                for t in range(MT)
            ]
            vaug = res.tile([128, SQ * 8 * VW], BF16, tag="vaug", name="vaug")
            wo_bf = [
                res.tile([128, H], BF16, tag=f"wob{t}", name=f"wob{t}")
                for t in range(MT)
            ]
            bq_sb = res.tile([128, MT], F32, tag="bqsb", name="bq_sb")

            # ---- constants / weights staging ----
            for m in range(MT):
                nc.sync.dma_start(
                    out=bq_sb[:, m : m + 1],
                    in_=bqd.ap()[m * 128 : (m + 1) * 128].rearrange(
                        "(p o) -> p o", o=1
                    ),
                )
            for t in range(MT):
                wos = rot.tile([128, H], F32, tag="wos", bufs=2, name=f"wos{t}")
                nc.sync.dma_start(out=wos, in_=wod.ap()[t * 128 : (t + 1) * 128, :])
                nc.vector.tensor_copy(wo_bf[t], wos)
            # ones columns of vaug (V slots are overwritten by the V proj)
            nc.vector.memset(vaug, 1.0)

            def load_w(wd):
                ws = []
                for kt in range(KT):
                    wt = rot.tile([128, GH], F32R, tag="w", bufs=CFG["w_bufs"], name=f"w{kt}")
                    nc.sync.dma_start(out=wt, in_=wd.ap()[kt * 128 : (kt + 1) * 128, :])
                    ws.append(wt)
                return ws

            def load_strips(xd, qc):
                xs = []
                for kt in range(KT):
                    st = rot.tile([128, 512], F32R, tag="xs", bufs=CFG["xs_bufs"], name=f"xs{kt}")
                    nc.sync.dma_start(
                        out=st,
                        in_=xd.ap()[
                            kt * 128 : (kt + 1) * 128, qc * 512 : (qc + 1) * 512
                        ],
                    )
                    xs.append(st)
                return xs

            # ---- V projection: vaug[kseq, head*65] (untransposed, bf16) ----
            wv_sb = load_w(wvd)
            for qc in range(NQC):
                xs = load_strips(xv, qc)
                for sql in range(4):
                    sq = qc * 4 + sql
                    ps = psmm.tile([128, 512], F32, tag="mm", name=f"psv{sq}")
                    for kt in range(KT):
                        nc.tensor.matmul(
                            ps,
                            lhsT=xs[kt][:, sql * 128 : (sql + 1) * 128],
                            rhs=wv_sb[kt],
                            start=(kt == 0),
                            stop=(kt == KT - 1),
                        )
                    base = sq * 8 * VW
                    for h in range(8):
                        nc.vector.tensor_copy(
                            vaug[:, base + h * VW : base + h * VW + D],
                            ps[:, h * D : (h + 1) * D],
                        )

            # ---- K projection: khT[m] = (xk @ wk)^T slice, f32r ----
            wk_sb = load_w(wkd)
            for qc in range(NQC):
                xs = load_strips(xk, qc)
                for m in range(MT):
                    ps = psmm.tile([128, 512], F32, tag="mm", name=f"psk{m}")
                    for kt in range(KT):
                        nc.tensor.matmul(
                            ps,
                            lhsT=wk_sb[kt][:, m * 128 : (m + 1) * 128],
                            rhs=xs[kt],
                            start=(kt == 0),
                            stop=(kt == KT - 1),
                        )
                    nc.vector.tensor_copy(khT[m][:, qc * 512 : (qc + 1) * 512], ps)

            # ---- Q projection (+ bias) ----
            wq_sb = load_w(wqd)

            def q_proj(qc):
                xs = load_strips(xq, qc)
                for m in range(MT):
                    ps = psmm.tile([128, 512], F32, tag="mm", name=f"psq{m}")
                    for kt in range(KT):
                        nc.tensor.matmul(
                            ps,
                            lhsT=wq_sb[kt][:, m * 128 : (m + 1) * 128],
                            rhs=xs[kt],
                            start=(kt == 0),
                            stop=(kt == KT - 1),
                        )
                    nc.vector.tensor_scalar(
                        qhT[m][:, qc * 512 : (qc + 1) * 512],
                        ps,
                        bq_sb[:, m : m + 1],
                        None,
                        OP.add,
                    )

            # ---- attention for head pair t over a 1024-wide query chunk ----
            def attention(t, qcp):
                q0 = qcp * 1024
                ps_o = [
                    pso.tile([VW, 1024], F32, tag="o", name=f"pso{hh}")
                    for hh in range(2)
                ]
                for kt in range(SQ):
                    # emit the two heads' score matmuls adjacently: they hit
                    # disjoint PE row groups (partitions 0-63 vs 64-127) and
                    # run concurrently on hardware (row tiling)
                    ps_ss = []
                    for hh in range(2):
                        hp = 64 * hh
                        ps_s = psmm.tile([128, 1024], F32, tag="mm", name="pss")
                        ps_ss.append(ps_s)
                        for qch in range(2):
                            nc.tensor.matmul(
                                ps_s[:, qch * 512 : (qch + 1) * 512],
                                lhsT=khT[t][hp : hp + 64, kt * 128 : (kt + 1) * 128],
                                rhs=qhT[t][
                                    hp : hp + 64, q0 + qch * 512 : q0 + (qch + 1) * 512
                                ],
                                start=True,
                                stop=True,
                            )
                    for hh in range(2):
                        h_abs = 2 * t + hh
                        pt_t = rot.tile([128, 1024], BF16, tag="pt", bufs=CFG["pt_bufs"], name="pt")
                        nc.scalar.activation(pt_t, ps_ss[hh], AF.Exp, scale=SCALE)
                        vbase = kt * 8 * VW + h_abs * VW
                        for qch in range(2):
                            nc.tensor.matmul(
                                ps_o[hh][:, qch * 512 : (qch + 1) * 512],
                                lhsT=vaug[:, vbase : vbase + VW],
                                rhs=pt_t[:, qch * 512 : (qch + 1) * 512],
                                start=(kt == 0),
                                stop=(kt == SQ - 1),
                            )
                # normalize by the ones-column sums; heads stack on partitions
                for hh in range(2):
                    rcp = rot.tile([VW, 1024], F32, tag="rcp", bufs=2, name="rcp")
                    nc.vector.reciprocal(rcp[D : D + 1, :], ps_o[hh][D : D + 1, :])
                    # broadcast the recip row to 64 partitions via a DRAM
                    # round-trip (DRAM source DMA supports 0-stride partition
                    # reads; SBUF source / gpsimd custom op do not work here)
                    sc = dsc.tile([1, 1024], F32, tag="sc", name="sc")
                    nc.sync.dma_start(out=sc, in_=rcp[D : D + 1, :])
                    bc = rot.tile([VW, 1024], F32, tag="bc", bufs=2, name="bc")
                    nc.sync.dma_start(
                        out=bc[0:64, :], in_=sc[0, :].partition_broadcast(64)
                    )
                    if hh == 0:
                        nc.vector.tensor_tensor(
                            oT[t][0:64, q0 : q0 + 1024],
                            ps_o[0][0:D, :],
                            bc[0:64, :],
                            OP.mult,
                        )
                    else:
                        # normalized h1 lands on partitions 0-63; DMA shifts it
                        # onto partitions 64-127 of the head-pair tile
                        otn = rot.tile([64, 1024], BF16, tag="otn", bufs=2, name="otn")
                        nc.vector.tensor_tensor(
                            otn, ps_o[1][0:D, :], bc[0:64, :], OP.mult
                        )
                        nc.sync.dma_start(
                            out=oT[t][64:128, q0 : q0 + 1024], in_=otn
                        )

            # ---- output projection for one 512-wide sequence chunk ----
            def out_proj(qcc):
                for m in range(H // 128):
                    ps = psmm.tile([128, 512], F32, tag="mm", name=f"pso{m}")
                    for t in range(MT):
                        nc.tensor.matmul(
                            ps,
                            lhsT=wo_bf[t][:, m * 128 : (m + 1) * 128],
                            rhs=oT[t][:, qcc * 512 : (qcc + 1) * 512],
                            start=(t == 0),
                            stop=(t == MT - 1),
                        )
                    osb = rot.tile([128, 512], F32, tag="osb", bufs=3, name="osb")
                    nc.vector.tensor_copy(osb, ps)
                    nc.sync.dma_start(
                        out=otd.ap()[m * 128 : (m + 1) * 128, qcc * 512 : (qcc + 1) * 512],
                        in_=osb,
                    )

            for qcp in range(2):
                q_proj(2 * qcp)
                q_proj(2 * qcp + 1)
                for t in range(MT):
                    attention(t, qcp)
                out_proj(2 * qcp)
                out_proj(2 * qcp + 1)

    nc.compile()
    return nc


def _get_nc():
    if "nc" not in _CACHE:
        _CACHE["nc"] = _build()
    return _CACHE["nc"]


def make_in_maps(q, k, v, wq, wk, wv, wo, bq):
    q = np.asarray(q, np.float32)
    k = np.asarray(k, np.float32)
    v = np.asarray(v, np.float32)
    in_maps = []
    for c in range(NCORES):
        g, b = divmod(c, B)
        sl = slice(g * GH, (g + 1) * GH)
        in_maps.append(
            {
                "xq": np.ascontiguousarray(q[b].T).astype(np.float16),
                "xk": np.ascontiguousarray(k[b].T).astype(np.float16),
                "xv": np.ascontiguousarray(v[b].T).astype(np.float16),
                "wq": np.ascontiguousarray(np.asarray(wq, np.float32)[:, sl]).astype(np.float16),
                "wk": np.ascontiguousarray(np.asarray(wk, np.float32)[:, sl]).astype(np.float16),
                "wv": np.ascontiguousarray(np.asarray(wv, np.float32)[:, sl]).astype(np.float16),
                "wo": np.ascontiguousarray(np.asarray(wo, np.float32)[sl, :]),
                "bq": np.ascontiguousarray(np.asarray(bq, np.float32)[sl]),
            }
        )
    return in_maps


def assemble(per_core_ot, bv, wo, bo):
    bo_eff = (
        np.asarray(bo, np.float32)
        + np.asarray(bv, np.float32) @ np.asarray(wo, np.float32)
    )
    out = np.empty((B, S, H), np.float32)
    for b in range(B):
        out[b] = per_core_ot[b].T + per_core_ot[B + b].T + bo_eff
    return out


def kernel(q, k, v, wq, bq, wk, bk, wv, bv, wo, bo, _trace=False):
    from concourse.bass_utils import run_bass_kernel_spmd

    nc = _get_nc()
    in_maps = make_in_maps(q, k, v, wq, wk, wv, wo, bq)
    res = run_bass_kernel_spmd(
        nc, in_maps, core_ids=list(range(NCORES)), trace=_trace
    )
    _CACHE["last_results"] = res
    outs = [res.results[c]["ot"] for c in range(NCORES)]
    return assemble(outs, bv, wo, bo)
